# revision 30
# baseline (speedup 1.0000x reference)
"""Trainium2 Bass kernel for masked additive (Bahdanau-style) attention.

Computes, for each batch b:
    ph    = h_b @ U                     (T, H)
    e     = tanh(ph + s_b @ W) @ v      (T,)
    e     = where(mask, e, -1e9)
    score = softmax(e)                  (T,)
    ctx   = sum_t score_t * h_b[t]      (D,)

Key observations baked into the kernel:
  * Masked timesteps contribute EXACTLY nothing: their energy is -1e9,
    exp underflows to exactly 0 in fp32, so score and context are
    untouched by them.  The host therefore COMPACTS h along T, keeping
    only unmasked timesteps (~half for this distribution).  The big
    h @ U matmul — the kernel's roofline — shrinks by ~2x.  This is
    exact, not an approximation.
  * Batches are assigned to (core, slot) by descending compacted
    length: slot k on every core holds the (8k..8k+7)-th longest
    batches, so the per-slot padded length hugs that slot's max
    instead of the global max.
  * After compaction every position before the short 256-wide tail
    tile is unmasked, so only the tail tile needs the mask fixup
    ((e+512)*m, with the tile's running max shifted back by 512 so all
    tiles combine on a common scale); other tiles run max/exp straight
    off PSUM and the mask DMA shrinks ~8x.  (Falls back to full-width
    masks for slots whose length spread exceeds the tail width.)
  * h^T tiles are laid out on the host as their exact SBUF images
    (tile-major), so every hT DMA moves 128 contiguous multi-KB lines:
    descriptor generation is ~10x cheaper than strided gathers.  The
    early DMA wave is kept lean (first tile split over two rings, U
    col-chunked, W in bf16) so the first matmul isn't stuck behind
    bulk traffic on bandwidth-fair DMA engines.
  * The big matmul (h @ U) is computed transposed: ph^T tiles with H on
    partitions, so the per-batch bias (s_b @ W) is a per-partition
    scalar that fuses into the tanh activation for free.
  * e is produced broadcast across all 128 partitions (the v-dot matmul
    uses a stationary operand whose 128 columns are all v), so the
    softmax runs at full 128-lane width with no partition reductions.
  * The softmax + context run flash-style per T-tile (local max/sum +
    fused multiply-accumulate over the resident h^T tile on the vector
    engine, rescaled at the end), so no h tile is ever touched twice.
  * h^T, U, W, s are fed to the PE in bf16 (full-rate, half the HBM
    traffic); everything downstream of the big matmul accumulates in
    fp32.  (fp8 was measured at 11% output error vs the 2% budget —
    dead.)

Sharding: pure data parallelism, 4 batches per core on 8 cores; no
collectives.  Host-side prep only shards, compacts and re-lays-out
inputs.  The module is compiled for the tile plan derived from the
actual mask (cached; any mask works).

Measured on trn2 (8 cores): ~166-170 us HW exec (NTFF, core 0) at the
2.4 GHz PE state, vs 308 us for the uncompacted baseline; rel err
(absmax/ref-absmax) 5.2e-3 vs the 2e-2 gate.  Breakdown: ~14 us
DMA-bound startup, ~138 us TensorMatrix busy (~130 us column roofline:
compacted main matmul + v-dot + proj), ~6 us exposed softmax/context
chain for the final tile, ~11 us framework drain.  NOTE the chip
sometimes sits in a sticky P0 power state (PE at 2.0 GHz, all matmul
slices exactly 1.2x longer) — identify it from slice durations before
comparing runs.
"""

import ml_dtypes
import numpy as np

import concourse.bass as bass
import concourse.tile as tile
from concourse import bacc, mybir
from concourse.bass_utils import run_bass_kernel_spmd
from concourse.masks import make_identity

F32 = mybir.dt.float32
BF16 = mybir.dt.bfloat16

B, T, D, H = 32, 2048, 1024, 1024
NCORES = 8
BL = B // NCORES          # batches (slots) per core
P = 128                   # partitions
KC = D // P               # 8 contraction chunks
MC = H // P               # 8 output-row chunks
AF = mybir.ActivationFunctionType
ALU = mybir.AluOpType


def _tile_sizes(T_pad):
    """Decompose T_pad into matmul tile widths: <=512 (PSUM bank limit),
    with a short 256 tail tile.  256 is the sweet spot: wide enough that
    each matmul (~109ns) still hides its LDWEIGHTS (~97ns), short enough
    to keep the serial softmax/context chain after the last matmul small."""
    if T_pad <= 512:
        return (T_pad,)
    rem = T_pad - 256
    n = -(-rem // 512)
    sizes = []
    left = rem
    for i in range(n):
        sz = -(-left // (n - i))
        sizes.append(sz)
        left -= sz
    return tuple(sizes + [256])


def _plan(mask):
    """(order, plans): batch assignment + per-slot tile plans.

    order[s*NCORES + c] = global batch index placed at slot s of core c
    (batches sorted by descending compacted length); plans[s] =
    (T_pad_s, tile_sizes_s, tail_only_mask_s).
    """
    mask = np.asarray(mask)
    L = (mask != 0).sum(axis=1).astype(np.int64)
    order = np.argsort(-L, kind="stable")
    plans = []
    for s in range(BL):
        Ls = int(L[order[s * NCORES]])
        Lmin = int(L[order[(s + 1) * NCORES - 1]])
        T_pad = max(8, -(-Ls // 8) * 8)
        tts = _tile_sizes(T_pad)
        tail_only = Lmin >= T_pad - tts[-1]
        plans.append((T_pad, tts, tail_only))
    return order, tuple(plans)


def _bcast_part(ap, parts=P):
    """Broadcast a 1-partition AP across `parts` partitions (step 0)."""
    return bass.AP(tensor=ap.tensor, offset=ap.offset, ap=[[0, parts]] + list(ap.ap))


def build_module(plans):
    T_pads = [pl[0] for pl in plans]
    tts_l = [pl[1] for pl in plans]
    tonly_l = [pl[2] for pl in plans]
    toff_l = [[sum(tts[:i]) for i in range(len(tts))] for tts in tts_l]
    mwid = [tts_l[s][-1] if tonly_l[s] else T_pads[s] for s in range(BL)]
    moff = [sum(mwid[:i]) for i in range(BL)]
    # flat per-(slot, tile) offsets into the tile-major hT input
    ht_base = []
    off = 0
    for s in range(BL):
        bases = []
        for ttw in tts_l[s]:
            bases.append(off)
            off += P * KC * ttw
        ht_base.append(bases)

    nc = bacc.Bacc(
        "TRN2",
        target_bir_lowering=False,
        debug=False,
        enable_asserts=False,
        num_devices=NCORES,
    )

    hT = nc.dram_tensor("hT", [off], BF16, kind="ExternalInput").ap()
    sT = nc.dram_tensor("sT", [D, BL], BF16, kind="ExternalInput").ap()
    maskf = nc.dram_tensor("maskf", [sum(mwid)], F32, kind="ExternalInput").ap()
    W = nc.dram_tensor("W", [D, H], BF16, kind="ExternalInput").ap()
    # U arrives as 8 per-mc-chunk SBUF images (p, kc, 128 cols each),
    # so every U DMA is 128 contiguous 2KB lines and group mc unblocks
    # as soon as its own chunk lands.
    U = nc.dram_tensor("U", [MC, P, KC, P], BF16, kind="ExternalInput").ap()
    v = nc.dram_tensor("v", [H, 1], F32, kind="ExternalInput").ap()
    out = nc.dram_tensor("out", [BL, P, KC], F32, kind="ExternalOutput").ap()

    def ht_ap(b, tt):
        o = ht_base[b][tt]
        ttw = tts_l[b][tt]
        return hT[o : o + P * KC * ttw].rearrange(
            "(p kc t) -> p kc t", p=P, kc=KC
        )

    with tile.TileContext(nc) as tc:
        with (
            tc.tile_pool(name="singles", bufs=1) as singles,
            tc.tile_pool(name="ht", bufs=5) as ht_pool,
            tc.tile_pool(name="mask", bufs=1) as mask_pool,
            tc.tile_pool(name="tanh", bufs=3) as tanh_pool,
            tc.tile_pool(name="p2", bufs=2) as p2_pool,
            tc.tile_pool(name="small", bufs=4) as small_pool,
            tc.tile_pool(name="ctx", bufs=2) as ctx_pool,
            tc.tile_pool(name="ps", bufs=6, space="PSUM") as ps_pool,
            tc.tile_pool(name="eps", bufs=2, space="PSUM") as e_pool,
        ):
            # ---- early DMA wave ------------------------------------------
            # Rings drain in issue order and the DMA engines share bandwidth
            # round-robin across rings, so the first-needed bytes go first:
            # gpsimd: ht(0,0) lo half, ht(0,1), W, ht(0,2), batches 1..3.
            # sync:   ht(0,0) hi half, U col-chunks (first chunk feeds the
            #         first matmul group), tail masks.
            # scalar: sT, v, outputs.
            ht00 = ht_pool.tile(
                [P, KC, tts_l[0][0]], BF16, tag="ht", name="ht_b0t0",
                padded_shape=(..., 512),
            )
            # sT + v go FIRST on the scalar ring (tiny, and the v_bc /
            # proj chains hang off them); its ht00 slice follows them.
            sT_sb = singles.tile([P, KC, BL], BF16)
            nc.scalar.dma_start(
                out=sT_sb, in_=sT.rearrange("(kc p) b -> p kc b", p=P)
            )
            v_col = singles.tile([P, MC], F32)
            nc.scalar.dma_start(out=v_col, in_=v.rearrange("(mc p) x -> p (mc x)", p=P))

            hT00_r = ht_ap(0, 0)
            nc.gpsimd.dma_start(out=ht00[:, 0:4, :], in_=hT00_r[:, 0:4, :])
            nc.sync.dma_start(out=ht00[:, 4:8, :], in_=hT00_r[:, 4:8, :])

            ht01 = ht_pool.tile(
                [P, KC, tts_l[0][1]], BF16, tag="ht", name="ht_b0t1",
                padded_shape=(..., 512),
            ) if len(tts_l[0]) > 1 else None
            if ht01 is not None:
                nc.gpsimd.dma_start(out=ht01, in_=ht_ap(0, 1))

            u_sb = singles.tile([P, MC, KC, P], BF16)
            for mc in range(MC):
                nc.sync.dma_start(out=u_sb[:, mc], in_=U[mc])

            w_sb = singles.tile([P, KC, H], BF16)
            W_r = W.rearrange("(kc p) n -> p kc n", p=P)
            for wc in range(4):
                nc.gpsimd.dma_start(
                    out=w_sb[:, 2 * wc : 2 * wc + 2, :],
                    in_=W_r[:, 2 * wc : 2 * wc + 2, :],
                )

            # v replicated into a (P, MC, P) stationary operand: for each
            # H-chunk mc, all 128 columns equal v[mc*128 + p].
            v_bc = singles.tile([P, MC, P], BF16)
            for mc in range(MC):
                nc.vector.memset(v_bc[:, mc, :], 0.0)
                nc.vector.tensor_scalar_add(
                    out=v_bc[:, mc, :],
                    in0=v_bc[:, mc, :],
                    scalar1=v_col[:, mc : mc + 1],
                )

            pos512 = singles.tile([P, 1], F32)
            nc.vector.memset(pos512, 512.0)

            # ---- emission helpers -----------------------------------------
            def emit_batch_dmas(b, pre=()):
                tts = tts_l[b]
                ht_tiles = list(pre)
                for tt in range(len(pre), len(tts)):
                    htt = ht_pool.tile(
                        [P, KC, tts[tt]], BF16, tag="ht", name=f"ht_b{b}t{tt}",
                        padded_shape=(..., 512),
                    )
                    nc.gpsimd.dma_start(out=htt, in_=ht_ap(b, tt))
                    ht_tiles.append(htt)
                mb_sb = mask_pool.tile(
                    [P, mwid[b]], F32, tag="m", name=f"mb{b}",
                    padded_shape=(..., max(mwid)),
                )
                nc.sync.dma_start(
                    out=mb_sb,
                    in_=_bcast_part(maskf[moff[b] : moff[b] + mwid[b]]),
                )
                return ht_tiles, mb_sb

            def emit_mains(b, tt, ht_tiles):
                pps = []
                for mc in range(MC):
                    pp = ps_pool.tile(
                        [P, tts_l[b][tt]], F32, tag="ps", name=f"pp{b}_{tt}_{mc}",
                        padded_shape=(..., 512),
                    )
                    for kc in range(KC):
                        nc.tensor.matmul(
                            pp,
                            lhsT=u_sb[:, mc, kc, :],
                            rhs=ht_tiles[tt][:, kc, :],
                            start=(kc == 0),
                            stop=(kc == KC - 1),
                        )
                    pps.append(pp)
                return pps

            def emit_tile_rest(b, tt, pps, ht_tiles, mb_sb, st):
                # tanh + v-dot, then the online-softmax tile pass.
                # Masked (= padded tail) positions: et = (e + 512) * m -> 0,
                # and exp(0 - max) underflows to exactly 0 in fp32 since the
                # valid entries sit at e + 512 ~ 512 (ulp(512)=6.1e-5 keeps
                # e's precision).  Tiles with no masked positions skip the
                # fixup and run max/exp straight off the PSUM accumulator;
                # the tail tile's running max is rescaled by -512 afterwards
                # so all tiles combine on a common scale.
                ttw = tts_l[b][tt]
                tonly = tonly_l[b]
                last = tt == len(tts_l[b]) - 1
                nmax, zs, part, scr = st
                e_ps = e_pool.tile(
                    [P, ttw], F32, tag="e", name=f"e{b}_{tt}",
                    padded_shape=(..., 512),
                )
                for mc in range(MC):
                    th = tanh_pool.tile(
                        [P, ttw], BF16, tag="th", name=f"th{b}_{tt}_{mc}",
                        padded_shape=(..., 512),
                    )
                    nc.scalar.activation(
                        out=th,
                        in_=pps[mc],
                        func=AF.Tanh,
                        bias=proj_sb[:, mc, b : b + 1],
                        scale=1.0,
                    )
                    nc.tensor.matmul(
                        e_ps,
                        lhsT=v_bc[:, mc, :],
                        rhs=th,
                        start=(mc == 0),
                        stop=(mc == MC - 1),
                    )
                if tonly and not last:
                    ex_in = e_ps
                else:
                    mo = toff_l[b][tt] - (T_pads[b] - mwid[b]) if tonly else toff_l[b][tt]
                    et = p2_pool.tile(
                        [P, ttw], F32, tag="et", name=f"et{b}_{tt}",
                        padded_shape=(..., 512),
                    )
                    nc.vector.scalar_tensor_tensor(
                        out=et,
                        in0=e_ps,
                        scalar=512.0,
                        in1=mb_sb[:, mo : mo + ttw],
                        op0=ALU.add,
                        op1=ALU.mult,
                    )
                    ex_in = et
                nc.vector.tensor_reduce(
                    out=nmax[:, tt : tt + 1],
                    in_=ex_in,
                    axis=mybir.AxisListType.X,
                    op=ALU.max,
                    negate=True,
                )
                # bf16 ex: scores lose ~0.4% relative, far inside the error
                # budget, and the scr accumulate below gets two 16-bit
                # operands (2x DVE throughput, shorter post-matmul tail).
                ex = p2_pool.tile(
                    [P, ttw], BF16, tag="ex", name=f"ex{b}_{tt}",
                    padded_shape=(..., 512),
                )
                nc.scalar.activation(
                    out=ex,
                    in_=ex_in,
                    func=AF.Exp,
                    bias=nmax[:, tt : tt + 1],
                    scale=1.0,
                    accum_out=zs[:, tt : tt + 1],
                )
                if tonly and last:
                    # bring the tail tile's -max back to the unshifted scale
                    nc.vector.tensor_scalar_add(
                        out=nmax[:, tt : tt + 1],
                        in0=nmax[:, tt : tt + 1],
                        scalar1=pos512,
                    )
                for dc in range(KC):
                    nc.vector.scalar_tensor_tensor(
                        out=scr[:, :ttw],
                        in0=ht_tiles[tt][:, dc, :],
                        scalar=1.0,
                        in1=ex,
                        op0=ALU.mult,
                        op1=ALU.mult,
                        accum_out=part[:, dc, tt : tt + 1],
                    )

            def emit_batch_tail(b, st):
                # combine tiles: f_i = exp(max_i - M) with global max M,
                # ctx = sum_i part_i f_i / sum_i z_i f_i  (all tiny tiles)
                NT = len(tts_l[b])
                nmax, zs, part, scr = st
                negM = small_pool.tile([P, 1], F32, tag="negM", name=f"nM{b}")
                nc.vector.tensor_reduce(
                    out=negM, in_=nmax, axis=mybir.AxisListType.X, op=ALU.min
                )
                f = small_pool.tile([P, NT], F32, tag="f", name=f"f{b}")
                nc.scalar.activation(
                    out=f, in_=nmax, func=AF.Exp, bias=negM, scale=-1.0
                )
                fz = small_pool.tile([P, NT], F32, tag="fz", name=f"fz{b}")
                zf = small_pool.tile([P, 1], F32, tag="zf", name=f"zf{b}")
                nc.vector.scalar_tensor_tensor(
                    out=fz,
                    in0=zs,
                    scalar=1.0,
                    in1=f,
                    op0=ALU.mult,
                    op1=ALU.mult,
                    accum_out=zf,
                )
                sinv = small_pool.tile([P, 1], F32, tag="sinv", name=f"si{b}")
                nc.vector.reciprocal(sinv, zf)
                for tt in range(NT):
                    nc.vector.tensor_scalar_mul(
                        out=part[:, :, tt : tt + 1],
                        in0=part[:, :, tt : tt + 1],
                        scalar1=f[:, tt : tt + 1],
                    )
                ctx = ctx_pool.tile([P, KC], F32, tag="ctx", name=f"cx{b}")
                nc.vector.tensor_reduce(
                    out=ctx, in_=part, axis=mybir.AxisListType.X, op=ALU.add
                )
                nc.vector.tensor_scalar_mul(out=ctx, in0=ctx, scalar1=sinv)
                nc.scalar.dma_start(out=out[b], in_=ctx)

            def batch_state(b):
                NT = len(tts_l[b])
                nmax = small_pool.tile(
                    [P, NT], F32, tag="nmax", name=f"nm{b}", padded_shape=(..., 8)
                )
                zs = small_pool.tile(
                    [P, NT], F32, tag="zs", name=f"zs{b}", padded_shape=(..., 8)
                )
                part = ctx_pool.tile(
                    [P, KC, NT], F32, tag="part", name=f"pt{b}", padded_shape=(..., 8)
                )
                scr = p2_pool.tile(
                    [P, max(tts_l[b])], F32, tag="scr", name=f"sc{b}",
                    padded_shape=(..., 512),
                )
                return nmax, zs, part, scr

            def emit_proj():
                # proj_s = s @ W (sT-stationary: the weight load is only
                # BL=4 columns), then 16 PE transposes of (4,128) chunks put
                # H on partitions for the tanh bias.  No DRAM round-trip.
                pnat = []
                for i in range(2):
                    pn = e_pool.tile([BL, 512], F32, tag="e", name=f"pnat{i}")
                    pnat.append(pn)
                for kc in range(KC):
                    for nh in range(2):
                        nc.tensor.matmul(
                            pnat[nh],
                            lhsT=sT_sb[:, kc, :],
                            rhs=w_sb[:, kc, nh * 512 : (nh + 1) * 512],
                            start=(kc == 0),
                            stop=(kc == KC - 1),
                        )
                pstg = singles.tile([BL, H], F32)
                for nh in range(2):
                    nc.vector.tensor_copy(
                        out=pstg[:, nh * 512 : (nh + 1) * 512], in_=pnat[nh]
                    )
                proj_sb = singles.tile([P, MC, BL], F32)
                for mc in range(MC):
                    tp = e_pool.tile([P, BL], F32, tag="e", name=f"tp{mc}")
                    nc.tensor.transpose(
                        tp, in_=pstg[:, mc * P : (mc + 1) * P], identity=identity4
                    )
                    nc.vector.tensor_copy(out=proj_sb[:, mc, :], in_=tp)
                return proj_sb

            identity4 = singles.tile([BL, BL], F32)
            make_identity(nc, identity4)

            # ---- pipeline -------------------------------------------------
            # Batch 0, tile 0's main matmuls are emitted BEFORE proj: they
            # only need hT(0,0) + the first U chunk, which land well before
            # all of W, so the PE warms up on dense main work while W
            # trickles in; the scheduler slots proj into the psum-runway
            # stall that follows.
            pre = (ht00,) if ht01 is None else (ht00, ht01)
            ht0, mb0 = emit_batch_dmas(0, pre=pre)
            st0 = batch_state(0)
            pps00 = emit_mains(0, 0, ht0)
            proj_sb = emit_proj()
            emit_tile_rest(0, 0, pps00, ht0, mb0, st0)
            for tt in range(1, len(tts_l[0])):
                pps = emit_mains(0, tt, ht0)
                emit_tile_rest(0, tt, pps, ht0, mb0, st0)
            emit_batch_tail(0, st0)

            for b in range(1, BL):
                ht_tiles, mb_sb = emit_batch_dmas(b)
                st = batch_state(b)
                for tt in range(len(tts_l[b])):
                    pps = emit_mains(b, tt, ht_tiles)
                    emit_tile_rest(b, tt, pps, ht_tiles, mb_sb, st)
                emit_batch_tail(b, st)

    nc.compile()
    return nc


_NC_CACHE = {}


def module_for(mask):
    _, plans = _plan(mask)
    if plans not in _NC_CACHE:
        _NC_CACHE[plans] = build_module(plans)
    return _NC_CACHE[plans]


def core_batches(mask, c):
    """Global batch indices assigned to core c, in slot order."""
    order, _ = _plan(mask)
    return [int(order[s * NCORES + c]) for s in range(BL)]


def core_in_map(s, h, mask, W, U, v, c):
    """Shard + compact + lay out the full inputs for core c."""
    mask = np.asarray(mask)
    order, plans = _plan(mask)
    h32 = np.asarray(h, np.float32)

    ht_parts = []
    mf_parts = []
    for sl in range(BL):
        gb = int(order[sl * NCORES + c])
        T_pad, tts, tail_only = plans[sl]
        idx = np.flatnonzero(mask[gb])
        L = idx.size
        hc = np.zeros((T_pad, D), dtype=ml_dtypes.bfloat16)
        if L:
            hc[:L] = h32[gb, idx]
        mf = np.zeros(T_pad, dtype=np.float32)
        mf[:L] = 1.0
        mf_parts.append(mf[T_pad - tts[-1] :] if tail_only else mf)
        to = 0
        for ttw in tts:
            seg = hc[to : to + ttw, :]              # (ttw, D)
            blk = np.ascontiguousarray(
                seg.T.reshape(KC, P, ttw).transpose(1, 0, 2)
            )                                       # (P, KC, ttw) SBUF image
            ht_parts.append(blk.ravel())
            to += ttw

    bs = [int(order[sl * NCORES + c]) for sl in range(BL)]
    return {
        "hT": np.concatenate(ht_parts),
        "sT": np.ascontiguousarray(
            np.asarray(s, np.float32)[0, bs].T.astype(ml_dtypes.bfloat16)
        ),
        "maskf": np.concatenate(mf_parts),
        "W": np.ascontiguousarray(np.asarray(W, np.float32).astype(ml_dtypes.bfloat16)),
        "U": np.ascontiguousarray(
            np.asarray(U, np.float32).astype(ml_dtypes.bfloat16)
            .reshape(KC, P, MC, P)        # (kc, p, mc, col)
            .transpose(2, 1, 0, 3)        # (mc, p, kc, col)
        ),
        "v": np.ascontiguousarray(np.asarray(v, np.float32).reshape(H, 1)),
    }


def out_to_ctx(out_c):
    """Device output (BL, P, KC) -> context rows (BL, D) with d = kc*P + p."""
    return np.asarray(out_c).transpose(0, 2, 1).reshape(BL, D)


def kernel(s, h, mask, W, U, v):
    mask = np.asarray(mask)
    order, _ = _plan(mask)
    in_maps = [core_in_map(s, h, mask, W, U, v, c) for c in range(NCORES)]
    nc = module_for(mask)
    res = run_bass_kernel_spmd(nc, in_maps, list(range(NCORES)))
    full = np.empty((B, D), dtype=np.float32)
    for c in range(NCORES):
        ctx = out_to_ctx(res.results[c]["out"])
        for sl in range(BL):
            full[int(order[sl * NCORES + c])] = ctx[sl]
    return full


# revision 31
# speedup vs baseline: 1.0037x; 1.0037x over previous
"""Trainium2 Bass kernel for masked additive (Bahdanau-style) attention.

Computes, for each batch b:
    ph    = h_b @ U                     (T, H)
    e     = tanh(ph + s_b @ W) @ v      (T,)
    e     = where(mask, e, -1e9)
    score = softmax(e)                  (T,)
    ctx   = sum_t score_t * h_b[t]      (D,)

Key observations baked into the kernel:
  * Masked timesteps contribute EXACTLY nothing: their energy is -1e9,
    exp underflows to exactly 0 in fp32, so score and context are
    untouched by them.  The host therefore COMPACTS h along T, keeping
    only unmasked timesteps (~half for this distribution).  The big
    h @ U matmul — the kernel's roofline — shrinks by ~2x.  This is
    exact, not an approximation.
  * Batches are assigned to (core, slot) by descending compacted
    length: slot k on every core holds the (8k..8k+7)-th longest
    batches, so the per-slot padded length hugs that slot's max
    instead of the global max.
  * After compaction every position before the short 256-wide tail
    tile is unmasked, so only the tail tile needs the mask fixup
    ((e+512)*m, with the tile's running max shifted back by 512 so all
    tiles combine on a common scale); other tiles run max/exp straight
    off PSUM and the mask DMA shrinks ~8x.  (Falls back to full-width
    masks for slots whose length spread exceeds the tail width.)
  * h^T tiles are laid out on the host as their exact SBUF images
    (tile-major), so every hT DMA moves 128 contiguous multi-KB lines:
    descriptor generation is ~10x cheaper than strided gathers.  The
    early DMA wave is kept lean (first tile split over two rings, U
    col-chunked, W in bf16) so the first matmul isn't stuck behind
    bulk traffic on bandwidth-fair DMA engines.
  * The big matmul (h @ U) is computed transposed: ph^T tiles with H on
    partitions, so the per-batch bias (s_b @ W) is a per-partition
    scalar that fuses into the tanh activation for free.
  * e is produced broadcast across all 128 partitions (the v-dot matmul
    uses a stationary operand whose 128 columns are all v), so the
    softmax runs at full 128-lane width with no partition reductions.
  * The softmax + context run flash-style per T-tile (local max/sum +
    fused multiply-accumulate over the resident h^T tile on the vector
    engine, rescaled at the end), so no h tile is ever touched twice.
  * h^T, U, W, s are fed to the PE in bf16 (full-rate, half the HBM
    traffic); everything downstream of the big matmul accumulates in
    fp32.  (fp8 was measured at 11% output error vs the 2% budget —
    dead.)

Sharding: pure data parallelism, 4 batches per core on 8 cores; no
collectives.  Host-side prep only shards, compacts and re-lays-out
inputs.  The module is compiled for the tile plan derived from the
actual mask (cached; any mask works).

Measured on trn2 (8 cores): ~166-170 us HW exec (NTFF, core 0) at the
2.4 GHz PE state, vs 308 us for the uncompacted baseline; rel err
(absmax/ref-absmax) 5.2e-3 vs the 2e-2 gate.  Breakdown: ~14 us
DMA-bound startup, ~138 us TensorMatrix busy (~130 us column roofline:
compacted main matmul + v-dot + proj), ~6 us exposed softmax/context
chain for the final tile, ~11 us framework drain.  NOTE the chip
sometimes sits in a sticky P0 power state (PE at 2.0 GHz, all matmul
slices exactly 1.2x longer) — identify it from slice durations before
comparing runs.
"""

import ml_dtypes
import numpy as np

import concourse.bass as bass
import concourse.tile as tile
from concourse import bacc, mybir
from concourse.bass_utils import run_bass_kernel_spmd
from concourse.masks import make_identity

F32 = mybir.dt.float32
BF16 = mybir.dt.bfloat16

B, T, D, H = 32, 2048, 1024, 1024
NCORES = 8
BL = B // NCORES          # batches (slots) per core
P = 128                   # partitions
KC = D // P               # 8 contraction chunks
MC = H // P               # 8 output-row chunks
AF = mybir.ActivationFunctionType
ALU = mybir.AluOpType


def _tile_sizes(T_pad):
    """Decompose T_pad into matmul tile widths: <=512 (PSUM bank limit),
    with a short 256 tail tile.  256 is the sweet spot: wide enough that
    each matmul (~109ns) still hides its LDWEIGHTS (~97ns), short enough
    to keep the serial softmax/context chain after the last matmul small."""
    if T_pad <= 512:
        return (T_pad,)
    rem = T_pad - 256
    n = -(-rem // 512)
    sizes = []
    left = rem
    for i in range(n):
        sz = -(-left // (n - i))
        sizes.append(sz)
        left -= sz
    return tuple(sizes + [256])


def _plan(mask):
    """(order, plans): batch assignment + per-slot tile plans.

    order[s*NCORES + c] = global batch index placed at slot s of core c
    (batches sorted by descending compacted length); plans[s] =
    (T_pad_s, tile_sizes_s, tail_only_mask_s).
    """
    mask = np.asarray(mask)
    L = (mask != 0).sum(axis=1).astype(np.int64)
    order = np.argsort(-L, kind="stable")
    plans = []
    for s in range(BL):
        Ls = int(L[order[s * NCORES]])
        Lmin = int(L[order[(s + 1) * NCORES - 1]])
        T_pad = max(8, -(-Ls // 8) * 8)
        tts = _tile_sizes(T_pad)
        tail_only = Lmin >= T_pad - tts[-1]
        plans.append((T_pad, tts, tail_only))
    return order, tuple(plans)


def _bcast_part(ap, parts=P):
    """Broadcast a 1-partition AP across `parts` partitions (step 0)."""
    return bass.AP(tensor=ap.tensor, offset=ap.offset, ap=[[0, parts]] + list(ap.ap))


def build_module(plans):
    T_pads = [pl[0] for pl in plans]
    tts_l = [pl[1] for pl in plans]
    tonly_l = [pl[2] for pl in plans]
    toff_l = [[sum(tts[:i]) for i in range(len(tts))] for tts in tts_l]
    mwid = [tts_l[s][-1] if tonly_l[s] else T_pads[s] for s in range(BL)]
    moff = [sum(mwid[:i]) for i in range(BL)]
    # flat per-(slot, tile) offsets into the tile-major hT input
    ht_base = []
    off = 0
    for s in range(BL):
        bases = []
        for ttw in tts_l[s]:
            bases.append(off)
            off += P * KC * ttw
        ht_base.append(bases)

    nc = bacc.Bacc(
        "TRN2",
        target_bir_lowering=False,
        debug=False,
        enable_asserts=False,
        num_devices=NCORES,
    )

    hT = nc.dram_tensor("hT", [off], BF16, kind="ExternalInput").ap()
    sT = nc.dram_tensor("sT", [D, BL], BF16, kind="ExternalInput").ap()
    maskf = nc.dram_tensor("maskf", [sum(mwid)], F32, kind="ExternalInput").ap()
    W = nc.dram_tensor("W", [D, H], BF16, kind="ExternalInput").ap()
    # U arrives as 8 per-mc-chunk SBUF images (p, kc, 128 cols each),
    # so every U DMA is 128 contiguous 2KB lines and group mc unblocks
    # as soon as its own chunk lands.
    U = nc.dram_tensor("U", [MC, P, KC, P], BF16, kind="ExternalInput").ap()
    v = nc.dram_tensor("v", [H, 1], F32, kind="ExternalInput").ap()
    out = nc.dram_tensor("out", [BL, P, KC], F32, kind="ExternalOutput").ap()

    def ht_ap(b, tt):
        o = ht_base[b][tt]
        ttw = tts_l[b][tt]
        return hT[o : o + P * KC * ttw].rearrange(
            "(p kc t) -> p kc t", p=P, kc=KC
        )

    with tile.TileContext(nc) as tc:
        with (
            tc.tile_pool(name="singles", bufs=1) as singles,
            tc.tile_pool(name="ht", bufs=5) as ht_pool,
            tc.tile_pool(name="mask", bufs=1) as mask_pool,
            tc.tile_pool(name="tanh", bufs=3) as tanh_pool,
            tc.tile_pool(name="p2", bufs=2) as p2_pool,
            tc.tile_pool(name="small", bufs=4) as small_pool,
            tc.tile_pool(name="ctx", bufs=2) as ctx_pool,
            tc.tile_pool(name="ps", bufs=6, space="PSUM") as ps_pool,
            tc.tile_pool(name="eps", bufs=2, space="PSUM") as e_pool,
        ):
            # ---- early DMA wave ------------------------------------------
            # Rings drain in issue order and the DMA engines share bandwidth
            # round-robin across rings, so the first-needed bytes go first:
            # gpsimd: ht(0,0) lo half, ht(0,1), W, ht(0,2), batches 1..3.
            # sync:   ht(0,0) hi half, U col-chunks (first chunk feeds the
            #         first matmul group), tail masks.
            # scalar: sT, v, outputs.
            ht00 = ht_pool.tile(
                [P, KC, tts_l[0][0]], BF16, tag="ht", name="ht_b0t0",
                padded_shape=(..., 512),
            )
            # sT + v go FIRST on the scalar ring (tiny, and the v_bc /
            # proj chains hang off them); its ht00 slice follows them.
            sT_sb = singles.tile([P, KC, BL], BF16)
            nc.scalar.dma_start(
                out=sT_sb, in_=sT.rearrange("(kc p) b -> p kc b", p=P)
            )
            v_col = singles.tile([P, MC], F32)
            nc.scalar.dma_start(out=v_col, in_=v.rearrange("(mc p) x -> p (mc x)", p=P))

            hT00_r = ht_ap(0, 0)
            nc.gpsimd.dma_start(out=ht00[:, 0:4, :], in_=hT00_r[:, 0:4, :])
            nc.sync.dma_start(out=ht00[:, 4:8, :], in_=hT00_r[:, 4:8, :])

            u_sb = singles.tile([P, MC, KC, P], BF16)
            for mc in range(MC):
                nc.sync.dma_start(out=u_sb[:, mc], in_=U[mc])

            # W ahead of ht(0,1) on the gpsimd ring: proj gates the first
            # tanh, and tile 0's PSUM banks can only recycle through tanh
            # once proj is done.  ht(0,1) still lands before tile 0's main
            # matmuls finish.
            w_sb = singles.tile([P, KC, H], BF16)
            W_r = W.rearrange("(kc p) n -> p kc n", p=P)
            for wc in range(4):
                nc.gpsimd.dma_start(
                    out=w_sb[:, 2 * wc : 2 * wc + 2, :],
                    in_=W_r[:, 2 * wc : 2 * wc + 2, :],
                )

            ht01 = ht_pool.tile(
                [P, KC, tts_l[0][1]], BF16, tag="ht", name="ht_b0t1",
                padded_shape=(..., 512),
            ) if len(tts_l[0]) > 1 else None
            if ht01 is not None:
                nc.gpsimd.dma_start(out=ht01, in_=ht_ap(0, 1))

            # v replicated into a (P, MC, P) stationary operand: for each
            # H-chunk mc, all 128 columns equal v[mc*128 + p].
            v_bc = singles.tile([P, MC, P], BF16)
            for mc in range(MC):
                nc.vector.memset(v_bc[:, mc, :], 0.0)
                nc.vector.tensor_scalar_add(
                    out=v_bc[:, mc, :],
                    in0=v_bc[:, mc, :],
                    scalar1=v_col[:, mc : mc + 1],
                )

            pos512 = singles.tile([P, 1], F32)
            nc.vector.memset(pos512, 512.0)

            # ---- emission helpers -----------------------------------------
            def emit_batch_dmas(b, pre=()):
                tts = tts_l[b]
                ht_tiles = list(pre)
                for tt in range(len(pre), len(tts)):
                    htt = ht_pool.tile(
                        [P, KC, tts[tt]], BF16, tag="ht", name=f"ht_b{b}t{tt}",
                        padded_shape=(..., 512),
                    )
                    nc.gpsimd.dma_start(out=htt, in_=ht_ap(b, tt))
                    ht_tiles.append(htt)
                mb_sb = mask_pool.tile(
                    [P, mwid[b]], F32, tag="m", name=f"mb{b}",
                    padded_shape=(..., max(mwid)),
                )
                nc.sync.dma_start(
                    out=mb_sb,
                    in_=_bcast_part(maskf[moff[b] : moff[b] + mwid[b]]),
                )
                return ht_tiles, mb_sb

            def emit_mains(b, tt, ht_tiles):
                pps = []
                for mc in range(MC):
                    pp = ps_pool.tile(
                        [P, tts_l[b][tt]], F32, tag="ps", name=f"pp{b}_{tt}_{mc}",
                        padded_shape=(..., 512),
                    )
                    for kc in range(KC):
                        nc.tensor.matmul(
                            pp,
                            lhsT=u_sb[:, mc, kc, :],
                            rhs=ht_tiles[tt][:, kc, :],
                            start=(kc == 0),
                            stop=(kc == KC - 1),
                        )
                    pps.append(pp)
                return pps

            def emit_tile_rest(b, tt, pps, ht_tiles, mb_sb, st):
                # tanh + v-dot, then the online-softmax tile pass.
                # Masked (= padded tail) positions: et = (e + 512) * m -> 0,
                # and exp(0 - max) underflows to exactly 0 in fp32 since the
                # valid entries sit at e + 512 ~ 512 (ulp(512)=6.1e-5 keeps
                # e's precision).  Tiles with no masked positions skip the
                # fixup and run max/exp straight off the PSUM accumulator;
                # the tail tile's running max is rescaled by -512 afterwards
                # so all tiles combine on a common scale.
                ttw = tts_l[b][tt]
                tonly = tonly_l[b]
                last = tt == len(tts_l[b]) - 1
                nmax, zs, part, scr = st
                e_ps = e_pool.tile(
                    [P, ttw], F32, tag="e", name=f"e{b}_{tt}",
                    padded_shape=(..., 512),
                )
                for mc in range(MC):
                    th = tanh_pool.tile(
                        [P, ttw], BF16, tag="th", name=f"th{b}_{tt}_{mc}",
                        padded_shape=(..., 512),
                    )
                    nc.scalar.activation(
                        out=th,
                        in_=pps[mc],
                        func=AF.Tanh,
                        bias=proj_sb[:, mc, b : b + 1],
                        scale=1.0,
                    )
                    nc.tensor.matmul(
                        e_ps,
                        lhsT=v_bc[:, mc, :],
                        rhs=th,
                        start=(mc == 0),
                        stop=(mc == MC - 1),
                    )
                if tonly and not last:
                    ex_in = e_ps
                else:
                    mo = toff_l[b][tt] - (T_pads[b] - mwid[b]) if tonly else toff_l[b][tt]
                    et = p2_pool.tile(
                        [P, ttw], F32, tag="et", name=f"et{b}_{tt}",
                        padded_shape=(..., 512),
                    )
                    nc.vector.scalar_tensor_tensor(
                        out=et,
                        in0=e_ps,
                        scalar=512.0,
                        in1=mb_sb[:, mo : mo + ttw],
                        op0=ALU.add,
                        op1=ALU.mult,
                    )
                    ex_in = et
                nc.vector.tensor_reduce(
                    out=nmax[:, tt : tt + 1],
                    in_=ex_in,
                    axis=mybir.AxisListType.X,
                    op=ALU.max,
                    negate=True,
                )
                # bf16 ex: scores lose ~0.4% relative, far inside the error
                # budget, and the scr accumulate below gets two 16-bit
                # operands (2x DVE throughput, shorter post-matmul tail).
                ex = p2_pool.tile(
                    [P, ttw], BF16, tag="ex", name=f"ex{b}_{tt}",
                    padded_shape=(..., 512),
                )
                nc.scalar.activation(
                    out=ex,
                    in_=ex_in,
                    func=AF.Exp,
                    bias=nmax[:, tt : tt + 1],
                    scale=1.0,
                    accum_out=zs[:, tt : tt + 1],
                )
                if tonly and last:
                    # bring the tail tile's -max back to the unshifted scale
                    nc.vector.tensor_scalar_add(
                        out=nmax[:, tt : tt + 1],
                        in0=nmax[:, tt : tt + 1],
                        scalar1=pos512,
                    )
                for dc in range(KC):
                    nc.vector.scalar_tensor_tensor(
                        out=scr[:, :ttw],
                        in0=ht_tiles[tt][:, dc, :],
                        scalar=1.0,
                        in1=ex,
                        op0=ALU.mult,
                        op1=ALU.mult,
                        accum_out=part[:, dc, tt : tt + 1],
                    )

            def emit_batch_tail(b, st):
                # combine tiles: f_i = exp(max_i - M) with global max M,
                # ctx = sum_i part_i f_i / sum_i z_i f_i  (all tiny tiles)
                NT = len(tts_l[b])
                nmax, zs, part, scr = st
                negM = small_pool.tile([P, 1], F32, tag="negM", name=f"nM{b}")
                nc.vector.tensor_reduce(
                    out=negM, in_=nmax, axis=mybir.AxisListType.X, op=ALU.min
                )
                f = small_pool.tile([P, NT], F32, tag="f", name=f"f{b}")
                nc.scalar.activation(
                    out=f, in_=nmax, func=AF.Exp, bias=negM, scale=-1.0
                )
                fz = small_pool.tile([P, NT], F32, tag="fz", name=f"fz{b}")
                zf = small_pool.tile([P, 1], F32, tag="zf", name=f"zf{b}")
                nc.vector.scalar_tensor_tensor(
                    out=fz,
                    in0=zs,
                    scalar=1.0,
                    in1=f,
                    op0=ALU.mult,
                    op1=ALU.mult,
                    accum_out=zf,
                )
                sinv = small_pool.tile([P, 1], F32, tag="sinv", name=f"si{b}")
                nc.vector.reciprocal(sinv, zf)
                for tt in range(NT):
                    nc.vector.tensor_scalar_mul(
                        out=part[:, :, tt : tt + 1],
                        in0=part[:, :, tt : tt + 1],
                        scalar1=f[:, tt : tt + 1],
                    )
                ctx = ctx_pool.tile([P, KC], F32, tag="ctx", name=f"cx{b}")
                nc.vector.tensor_reduce(
                    out=ctx, in_=part, axis=mybir.AxisListType.X, op=ALU.add
                )
                nc.vector.tensor_scalar_mul(out=ctx, in0=ctx, scalar1=sinv)
                nc.scalar.dma_start(out=out[b], in_=ctx)

            def batch_state(b):
                NT = len(tts_l[b])
                nmax = small_pool.tile(
                    [P, NT], F32, tag="nmax", name=f"nm{b}", padded_shape=(..., 8)
                )
                zs = small_pool.tile(
                    [P, NT], F32, tag="zs", name=f"zs{b}", padded_shape=(..., 8)
                )
                part = ctx_pool.tile(
                    [P, KC, NT], F32, tag="part", name=f"pt{b}", padded_shape=(..., 8)
                )
                scr = p2_pool.tile(
                    [P, max(tts_l[b])], F32, tag="scr", name=f"sc{b}",
                    padded_shape=(..., 512),
                )
                return nmax, zs, part, scr

            def emit_proj():
                # proj_s = s @ W (sT-stationary: the weight load is only
                # BL=4 columns), then 16 PE transposes of (4,128) chunks put
                # H on partitions for the tanh bias.  No DRAM round-trip.
                pnat = []
                for i in range(2):
                    pn = e_pool.tile([BL, 512], F32, tag="e", name=f"pnat{i}")
                    pnat.append(pn)
                for kc in range(KC):
                    for nh in range(2):
                        nc.tensor.matmul(
                            pnat[nh],
                            lhsT=sT_sb[:, kc, :],
                            rhs=w_sb[:, kc, nh * 512 : (nh + 1) * 512],
                            start=(kc == 0),
                            stop=(kc == KC - 1),
                        )
                pstg = singles.tile([BL, H], F32)
                for nh in range(2):
                    nc.vector.tensor_copy(
                        out=pstg[:, nh * 512 : (nh + 1) * 512], in_=pnat[nh]
                    )
                proj_sb = singles.tile([P, MC, BL], F32)
                for mc in range(MC):
                    tp = e_pool.tile([P, BL], F32, tag="e", name=f"tp{mc}")
                    nc.tensor.transpose(
                        tp, in_=pstg[:, mc * P : (mc + 1) * P], identity=identity4
                    )
                    nc.vector.tensor_copy(out=proj_sb[:, mc, :], in_=tp)
                return proj_sb

            identity4 = singles.tile([BL, BL], F32)
            make_identity(nc, identity4)

            # ---- pipeline -------------------------------------------------
            # Batch 0, tile 0's main matmuls are emitted BEFORE proj: they
            # only need hT(0,0) + the first U chunk, which land well before
            # all of W, so the PE warms up on dense main work while W
            # trickles in; the scheduler slots proj into the psum-runway
            # stall that follows.
            pre = (ht00,) if ht01 is None else (ht00, ht01)
            ht0, mb0 = emit_batch_dmas(0, pre=pre)
            st0 = batch_state(0)
            pps00 = emit_mains(0, 0, ht0)
            proj_sb = emit_proj()
            emit_tile_rest(0, 0, pps00, ht0, mb0, st0)
            for tt in range(1, len(tts_l[0])):
                pps = emit_mains(0, tt, ht0)
                emit_tile_rest(0, tt, pps, ht0, mb0, st0)
            emit_batch_tail(0, st0)

            for b in range(1, BL):
                ht_tiles, mb_sb = emit_batch_dmas(b)
                st = batch_state(b)
                for tt in range(len(tts_l[b])):
                    pps = emit_mains(b, tt, ht_tiles)
                    emit_tile_rest(b, tt, pps, ht_tiles, mb_sb, st)
                emit_batch_tail(b, st)

    nc.compile()
    return nc


_NC_CACHE = {}


def module_for(mask):
    _, plans = _plan(mask)
    if plans not in _NC_CACHE:
        _NC_CACHE[plans] = build_module(plans)
    return _NC_CACHE[plans]


def core_batches(mask, c):
    """Global batch indices assigned to core c, in slot order."""
    order, _ = _plan(mask)
    return [int(order[s * NCORES + c]) for s in range(BL)]


def core_in_map(s, h, mask, W, U, v, c):
    """Shard + compact + lay out the full inputs for core c."""
    mask = np.asarray(mask)
    order, plans = _plan(mask)
    h32 = np.asarray(h, np.float32)

    ht_parts = []
    mf_parts = []
    for sl in range(BL):
        gb = int(order[sl * NCORES + c])
        T_pad, tts, tail_only = plans[sl]
        idx = np.flatnonzero(mask[gb])
        L = idx.size
        hc = np.zeros((T_pad, D), dtype=ml_dtypes.bfloat16)
        if L:
            hc[:L] = h32[gb, idx]
        mf = np.zeros(T_pad, dtype=np.float32)
        mf[:L] = 1.0
        mf_parts.append(mf[T_pad - tts[-1] :] if tail_only else mf)
        to = 0
        for ttw in tts:
            seg = hc[to : to + ttw, :]              # (ttw, D)
            blk = np.ascontiguousarray(
                seg.T.reshape(KC, P, ttw).transpose(1, 0, 2)
            )                                       # (P, KC, ttw) SBUF image
            ht_parts.append(blk.ravel())
            to += ttw

    bs = [int(order[sl * NCORES + c]) for sl in range(BL)]
    return {
        "hT": np.concatenate(ht_parts),
        "sT": np.ascontiguousarray(
            np.asarray(s, np.float32)[0, bs].T.astype(ml_dtypes.bfloat16)
        ),
        "maskf": np.concatenate(mf_parts),
        "W": np.ascontiguousarray(np.asarray(W, np.float32).astype(ml_dtypes.bfloat16)),
        "U": np.ascontiguousarray(
            np.asarray(U, np.float32).astype(ml_dtypes.bfloat16)
            .reshape(KC, P, MC, P)        # (kc, p, mc, col)
            .transpose(2, 1, 0, 3)        # (mc, p, kc, col)
        ),
        "v": np.ascontiguousarray(np.asarray(v, np.float32).reshape(H, 1)),
    }


def out_to_ctx(out_c):
    """Device output (BL, P, KC) -> context rows (BL, D) with d = kc*P + p."""
    return np.asarray(out_c).transpose(0, 2, 1).reshape(BL, D)


def kernel(s, h, mask, W, U, v):
    mask = np.asarray(mask)
    order, _ = _plan(mask)
    in_maps = [core_in_map(s, h, mask, W, U, v, c) for c in range(NCORES)]
    nc = module_for(mask)
    res = run_bass_kernel_spmd(nc, in_maps, list(range(NCORES)))
    full = np.empty((B, D), dtype=np.float32)
    for c in range(NCORES):
        ctx = out_to_ctx(res.results[c]["out"])
        for sl in range(BL):
            full[int(order[sl * NCORES + c])] = ctx[sl]
    return full
